# revision 1
# baseline (speedup 1.0000x reference)
"""Trainium2 Bass kernel for nn_DGN6 (gnn_message_passing).

Reference computation (per batch element, 3 rounds with K = 4, 8, 16):
    S = h @ h.T; causal top-K neighbors per row; msg = masked mean of
    neighbor rows; h = mom*h + (1-mom)*gelu((mix*h + (1-mix)*msg)*gain + bias)
Output: (h - x) * scale.

Distribution: data-parallel over B (2 batches), each batch's rows split
over 4 cores (8 cores total).  Core c handles batch c//4 and, within it,
4 row-blocks of 128 rows: blocks {cc + 4k, k=0..3} where cc = c%4 ("slot"
k holds block cc + 4k).  Every core thus runs an IDENTICAL instruction
stream (required: one SPMD program), with slot k's causal j-window padded
to the per-slot maximum g(k) = 4*(k+1) 128-chunks; per-core differences
live entirely in input DATA (causal masks, row data, per-row weights).

Per round, per slot:
  scores  S_blk = hT_mine[slot].T @ hT  (fp32 matmuls, PSUM accumulated over
          8 d-chunks), masked-copied to SBUF with an additive causal mask.
          fp32 is required: top-k selection vs the fp32 reference flips on
          ~1e-4-level score noise (measured: f32r ~1.6e-4, bf16 ~2.5e-3 rel).
  top-k   nc.vector.max (top-8) [+ match_replace + max again for K=16]
          -> threshold th = K-th largest; th' = max(th, -1e29).
  mask    M01 = (S_masked >= th') as bf16 0/1 -> PE-transposed per
          128-chunk to M01^T.
  agg     msg_raw = M01 @ (h_hi + h_lo)   (bf16 split of h, exact 0/1 mask,
          fp32 PSUM accumulate; contraction over causal j-chunks).
  update  u = (h + msg_raw*w2) * (gain*mix) + bias;  w2 = (1-mix)/(mix*cnt)
          h' = mom*h + (1-mom)*gelu(u)    (exact erf gelu on ACT engine)
  layout  h' rows are PE-transposed into hT_mine; h' is cast to bf16 hi/lo.
Between rounds, h is exchanged with two pipelined AllGathers per round
(groups of 4, payload per slot packed as [hT-cols f32 | hi bf16 | lo bf16]):
phase 0 (slots 0-1) fires mid-round and overlaps slots 2-3; phase 1 fires
at the round boundary and overlaps the next round's slots 0-1, which by
construction only read phase-0 columns/chunks.  Back-DMAs are emitted
after all of the round's readers (Tile orders the WAR) and routed so they
never block a FIFO another slot is waiting on.  Round 3 folds the
momentum update and the final (h-x)*scale directly into the output rows
(no gather needed).

All scalar parameters (sigmoid/softplus of the inputs) are applied on the
host into small input tensors, so the device program depends only on
shapes.
"""

import math
import numpy as np

import concourse.bacc as bacc
import concourse.bass as bass
import concourse.mybir as mybir
import concourse.tile as tile
from concourse import bass_utils
from concourse.alu_op_type import AluOpType

F32 = mybir.dt.float32
BF16 = mybir.dt.bfloat16
AF = mybir.ActivationFunctionType
BF16_NP = mybir.dt.np(BF16)

NEG_MASK = -3.0e38  # additive causal mask value (bf16-representable)
NEG_CLAMP = -1.0e29  # threshold clamp: above mask, below any real score

K_SCHEDULE = (4, 8, 16)


class Cfg:
    def __init__(self, B=2, T=2048, D=1024, G=4, S=4):
        self.B, self.T, self.D, self.G, self.S = B, T, D, G, S
        self.P = 128
        self.DC = D // 128          # d-chunks
        self.NBLK = G * S           # row blocks per batch
        assert self.NBLK * 128 == T
        self.n_cores = B * G
        self.R = len(K_SCHEDULE)
        # slot k covers j-chunks [0, g(k)); block of core cc in slot k is cc + G*k
        self.g = [G * (k + 1) for k in range(S)]
        self.OFF = [128 * sum(self.g[:k]) for k in range(S)]  # mask free-dim offsets
        self.MTOT = 128 * sum(self.g)
        self.groups = [list(range(b * G, (b + 1) * G)) for b in range(B)]


def build_program(cfg: Cfg):
    """Build the single SPMD Bass/Tile program (identical on all cores)."""
    nc = bacc.Bacc(
        "TRN2", target_bir_lowering=False, debug=False,
        num_devices=cfg.n_cores,
    )
    P, D, T, DC, S, G, R = cfg.P, cfg.D, cfg.T, cfg.DC, cfg.S, cfg.G, cfg.R

    # ---- I/O ----
    i_hT = nc.dram_tensor("i_hT", [P, DC * T], F32, kind="ExternalInput")
    i_hhi = nc.dram_tensor("i_hhi", [P, cfg.NBLK * D], BF16, kind="ExternalInput")
    i_hlo = nc.dram_tensor("i_hlo", [P, cfg.NBLK * D], BF16, kind="ExternalInput")
    i_myh = nc.dram_tensor("i_myh", [P, S * D], F32, kind="ExternalInput")
    i_hTm = nc.dram_tensor("i_hTm", [P, S * D], F32, kind="ExternalInput")
    i_msk = nc.dram_tensor("i_msk", [P, cfg.MTOT], BF16, kind="ExternalInput")
    i_xs = nc.dram_tensor("i_xs", [P, S * D], F32, kind="ExternalInput")
    i_gm = nc.dram_tensor("i_gm", [R, P, D], F32, kind="ExternalInput")
    i_bb = nc.dram_tensor("i_bb", [R, P, D], F32, kind="ExternalInput")
    i_w2 = nc.dram_tensor("i_w2", [P, R * S], F32, kind="ExternalInput")
    # per-partition scalar params: col 0 = mom, 1 = s*(1-mom), 2 = s*mom, 3 = 1-mom
    i_sc = nc.dram_tensor("i_sc", [P, 4], F32, kind="ExternalInput")
    i_idf = nc.dram_tensor("i_idf", [P, 128], F32, kind="ExternalInput")
    i_idb = nc.dram_tensor("i_idb", [P, 128], BF16, kind="ExternalInput")
    o_out = nc.dram_tensor("o_out", [S, P, D], F32, kind="ExternalOutput")

    NH = D // 512  # 512-wide halves of D
    with tile.TileContext(nc) as tc:
        with (
            tc.tile_pool(name="const", bufs=1) as const,
            tc.tile_pool(name="work", bufs=2) as work,
            tc.tile_pool(name="psum", bufs=2, space="PSUM") as psum,
            tc.tile_pool(name="dram", bufs=1, space="DRAM") as dram,
        ):
            # ---- persistent state ----
            hT = const.tile([P, DC * T], F32, name="hT")
            hhi = const.tile([P, cfg.NBLK * D], BF16, name="hhi")
            hlo = const.tile([P, cfg.NBLK * D], BF16, name="hlo")
            myh = const.tile([P, S * D], F32, name="myh")
            hTm = const.tile([P, S * D], F32, name="hTm")
            w2t = const.tile([P, R * S], F32, name="w2t")
            sct = const.tile([P, 4], F32, name="sct")
            idf = const.tile([P, 128], F32, name="idf")
            idb = const.tile([P, 128], BF16, name="idb")

            # initial loads.  Slot-0-critical pieces go first on the sync
            # (HWDGE) queue; the remaining bulk state rides the Pool queue in
            # need-order so round-1 compute starts within ~10us.
            hTv = hT.rearrange("p (c j) -> p c j", c=DC)
            iTv = i_hT[:].rearrange("p (c j) -> p c j", c=DC)
            j1_0 = cfg.g[0]
            # strict first-use order: slot-0 scores need hTm[slot 0] + hT w0;
            # then the mask-transpose identity, slot-0's agg operands, the
            # remaining hTm slots, and only then the elementwise-phase consts.
            nc.sync.dma_start(hTm[:, 0:D], i_hTm[:, 0:D])
            nc.sync.dma_start(hTv[:, :, 0:j1_0 * 128], iTv[:, :, 0:j1_0 * 128])
            nc.sync.dma_start(idb[:], i_idb[:])
            nc.sync.dma_start(hhi[:, 0:j1_0 * D], i_hhi[:, 0:j1_0 * D])
            nc.sync.dma_start(hlo[:, 0:j1_0 * D], i_hlo[:, 0:j1_0 * D])
            nc.sync.dma_start(hTm[:, D:S * D], i_hTm[:, D:S * D])
            nc.sync.dma_start(sct[:], i_sc[:])
            nc.sync.dma_start(w2t[:], i_w2[:])
            nc.sync.dma_start(myh[:], i_myh[:])
            nc.sync.dma_start(idf[:], i_idf[:])
            for k in range(1, S):  # bulk, in the order later slots need it
                j0, j1 = cfg.g[k - 1], cfg.g[k]
                nc.gpsimd.dma_start(hTv[:, :, j0 * 128:j1 * 128],
                                    iTv[:, :, j0 * 128:j1 * 128])
                nc.gpsimd.dma_start(hhi[:, j0 * D:j1 * D], i_hhi[:, j0 * D:j1 * D])
                nc.gpsimd.dma_start(hlo[:, j0 * D:j1 * D], i_hlo[:, j0 * D:j1 * D])

            ap_mom = sct[:, 0:1]
            ap_s1m = sct[:, 1:2]
            ap_sm = sct[:, 2:3]
            ap_1m = sct[:, 3:4]

            # per-round, per-phase AllGather buffers (DRAM).
            # One packed payload per slot: [hT-cols f32 (D) | hi bf16 (D/2 f32) | lo bf16 (D/2 f32)]
            # Phase p covers slots {2p, 2p+1}; fired as soon as those slots finish,
            # so the collective overlaps the remaining compute.
            NPH = S // 2  # phases per round
            PW = D + D // 2 + D // 2  # packed payload width in f32 elems
            ag_in = [[dram.tile([2, P, PW], F32, name=f"ag_in{r}_{p}", tag=f"agi{r}_{p}")
                      for p in range(NPH)] for r in range(R - 1)]
            ag_out = [[dram.tile([2 * G, P, PW], F32, name=f"ag_out{r}_{p}", tag=f"ago{r}_{p}")
                       for p in range(NPH)] for r in range(R - 1)]

            for r in range(R):
                K = K_SCHEDULE[r]
                # per-round small loads ride the ACT HWDGE queue (only gelus
                # live there) so round boundaries never queue them behind the
                # AllGather back-DMAs on SP.
                gmt = work.tile([P, D], F32, tag="gmt", bufs=1, name=f"gmt{r}")
                nc.scalar.dma_start(gmt[:], i_gm[r])
                bbt = work.tile([P, D], F32, tag="bbt", bufs=1, name=f"bbt{r}")
                nc.scalar.dma_start(bbt[:], i_bb[r])

                # propagation of slot k-1 is deferred until slot k's scores are
                # queued: the h'-transposes wait on the DVE/ACT elementwise
                # chain, and on the strict PE FIFO they would otherwise stall
                # the next slot's (independent) score matmuls.
                pending_prop = None

                for k in range(S):
                    g = cfg.g[k]
                    W = g * 128
                    msk = work.tile([P, W], BF16, tag="msk", bufs=1, name=f"msk_{r}_{k}")
                    nc.scalar.dma_start(msk[:], i_msk[:, cfg.OFF[k]:cfg.OFF[k] + W])
                    # ---- scores: S_blk = hT_mine[slot k].T @ hT[:, :W] ----
                    sc = work.tile([P, W], F32, tag="sc", bufs=2, name=f"sc_{r}_{k}")
                    for w0 in range(0, W, 512):
                        n = min(512, W - w0)
                        ps = psum.tile([P, 512], F32, tag="ps_sc", bufs=3, name=f"ps_{r}_{k}_{w0}")
                        for dc in range(DC):
                            nc.tensor.matmul(
                                ps[:, :n],
                                hTm[:, (k * DC + dc) * 128:(k * DC + dc + 1) * 128],
                                hT[:, dc * T + w0: dc * T + w0 + n],
                                start=(dc == 0), stop=(dc == DC - 1),
                            )
                        # masked copy PSUM -> SBUF: sc = S + mask  (mask 0 / -3e38)
                        nc.vector.scalar_tensor_tensor(
                            sc[:, w0:w0 + n], ps[:, :n], 1.0,
                            msk[:, w0:w0 + n],
                            AluOpType.mult, AluOpType.add,
                        )
                    if pending_prop is not None:
                        pending_prop()
                        pending_prop = None
                    # ---- top-K threshold ----
                    mx = work.tile([P, 8], F32, tag="mx", name=f"mx_{r}_{k}")
                    nc.vector.max(out=mx[:], in_=sc[:])
                    if K <= 8:
                        th_src = mx[:, K - 1:K]
                    else:
                        scr = work.tile([P, W], F32, tag="sc", bufs=2, name=f"scr_{r}_{k}")
                        nc.vector.match_replace(out=scr[:], in_to_replace=mx[:],
                                                in_values=sc[:], imm_value=NEG_MASK)
                        mx2 = work.tile([P, 8], F32, tag="mx2", name=f"mx2_{r}_{k}")
                        nc.vector.max(out=mx2[:], in_=scr[:])
                        th_src = mx2[:, K - 9:K - 8]
                    th = work.tile([P, 1], F32, tag="th", name=f"th_{r}_{k}")
                    nc.vector.tensor_scalar_max(th[:], th_src, NEG_CLAMP)
                    # ---- 0/1 neighbor mask (bf16) ----
                    m01 = work.tile([P, W], BF16, tag="m01", bufs=1, name=f"m01_{r}_{k}")
                    nc.vector.tensor_scalar(m01[:], sc[:], th[:], None, AluOpType.is_ge)
                    # ---- transpose mask chunks ----
                    mts = []
                    for jc in range(g):
                        pt = psum.tile([P, 128], BF16, tag="ps_tr", name=f"pt_{r}_{k}_{jc}")
                        nc.tensor.transpose(pt[:], m01[:, jc * 128:(jc + 1) * 128], idb[:])
                        mt = work.tile([P, 128], BF16, tag="mt", bufs=17, name=f"mt_{r}_{k}_{jc}")
                        nc.vector.tensor_copy(mt[:], pt[:])
                        mts.append(mt)
                    # ---- aggregate: msg_raw = M01 @ (hi + lo) ----
                    pss = []
                    for h_ in range(NH):
                        pa = psum.tile([P, 512], F32, tag="ps_ag", bufs=3, name=f"pa_{r}_{k}_{h_}")
                        for jc in range(g):
                            nc.tensor.matmul(pa[:], mts[jc][:],
                                             hhi[:, jc * D + h_ * 512: jc * D + h_ * 512 + 512],
                                             start=(jc == 0), stop=False)
                            nc.tensor.matmul(pa[:], mts[jc][:],
                                             hlo[:, jc * D + h_ * 512: jc * D + h_ * 512 + 512],
                                             start=False, stop=(jc == g - 1))
                        pss.append(pa)
                    # ---- elementwise update (in-place chain in t1) ----
                    w2ap = w2t[:, r * S + k: r * S + k + 1]
                    for h_ in range(NH):
                        sl = slice(k * D + h_ * 512, k * D + (h_ + 1) * 512)
                        hsl = slice(h_ * 512, (h_ + 1) * 512)
                        t1 = work.tile([P, 512], F32, tag="t1", name=f"t1_{r}_{k}_{h_}")
                        nc.vector.scalar_tensor_tensor(
                            t1[:], pss[h_][:], w2ap, myh[:, sl],
                            AluOpType.mult, AluOpType.add)
                        nc.vector.tensor_mul(t1[:], t1[:], gmt[:, hsl])
                        nc.vector.tensor_add(t1[:], t1[:], bbt[:, hsl])
                        gl = work.tile([P, 512], F32, tag="gl", name=f"gl_{r}_{k}_{h_}")
                        nc.scalar.activation(gl[:], t1[:], AF.Gelu)
                        if r < R - 1:
                            nc.vector.tensor_scalar_mul(gl[:], gl[:], ap_1m)
                            # in-place: myh = mom*myh + (1-mom)*gelu
                            nc.vector.scalar_tensor_tensor(
                                myh[:, sl], myh[:, sl], ap_mom, gl[:],
                                AluOpType.mult, AluOpType.add)
                        else:
                            xst = work.tile([P, 512], F32, tag="hi_t", bufs=1, name=f"xst_{r}_{k}_{h_}")
                            nc.sync.dma_start(xst[:], i_xs[:, sl])
                            # gl <- s*(1-mom)*gelu - s*x
                            nc.vector.scalar_tensor_tensor(
                                gl[:], gl[:], ap_s1m, xst[:],
                                AluOpType.mult, AluOpType.subtract)
                            # t1 <- s*mom*h + gl
                            nc.vector.scalar_tensor_tensor(
                                t1[:], myh[:, sl], ap_sm, gl[:],
                                AluOpType.mult, AluOpType.add)
                            nc.sync.dma_start(o_out[k, :, hsl], t1[:])
                    # ---- propagate h' into transposed layout + bf16 split ----
                    if r < R - 1:
                        def _prop(kk=k, rr=r):
                            for dc in range(DC):
                                pt2 = psum.tile([P, 128], F32, tag="ps_tr", name=f"pt2_{rr}_{kk}_{dc}")
                                nc.tensor.transpose(pt2[:], myh[:, kk * D + dc * 128: kk * D + (dc + 1) * 128], idf[:])
                                nc.vector.tensor_copy(hTm[:, (kk * DC + dc) * 128:(kk * DC + dc + 1) * 128], pt2[:])
                            hi_t = work.tile([P, D], BF16, tag="hi_t", bufs=1, name=f"hi_{rr}_{kk}")
                            nc.vector.tensor_copy(hi_t[:], myh[:, kk * D:(kk + 1) * D])
                            p_, q_ = divmod(kk, 2)
                            agp = ag_in[rr][p_]
                            nc.sync.dma_start(agp[q_, :, 0:D], hTm[:, kk * D:(kk + 1) * D])
                            nc.sync.dma_start(agp[q_, :, D:D + D // 2].bitcast(BF16), hi_t[:])
                            # hi_t <- lo = myh - bf16(myh)  (after the hi DMA drains)
                            nc.vector.tensor_sub(hi_t[:], myh[:, kk * D:(kk + 1) * D], hi_t[:])
                            nc.sync.dma_start(agp[q_, :, D + D // 2:PW].bitcast(BF16), hi_t[:])
                            # fire the phase collective as soon as its 2 slots are
                            # staged; it only touches DRAM ag buffers, so it
                            # overlaps the remaining slots' compute.
                            if q_ == 1 and p_ < NPH - 1:
                                nc.gpsimd.collective_compute(
                                    "AllGather", AluOpType.bypass, replica_groups=cfg.groups,
                                    ins=[ag_in[rr][p_].opt()], outs=[ag_out[rr][p_].opt()])
                        pending_prop = _prop

                if pending_prop is not None:
                    pending_prop()
                    pending_prop = None

                # ---- round boundary: last phase collective + all back-DMAs ----
                # Emitted after the slot loop so Tile orders them after this
                # round's readers of hT/hhi/hlo (they carry next round's state).
                # Early phases' backs ride SP (their waits resolve mid-slot-3);
                # the last phase's ride Pool behind its own collective so they
                # never block SP's next-round loads.
                if r < R - 1:
                    nc.gpsimd.collective_compute(
                        "AllGather", AluOpType.bypass, replica_groups=cfg.groups,
                        ins=[ag_in[r][NPH - 1].opt()], outs=[ag_out[r][NPH - 1].opt()])

                    def _backs(p_, eng):
                        # hT columns first (next round's slot-0 scores need the
                        # lowest blocks first), then the bf16 rows for agg.
                        for idx in range(2 * G):
                            rnk, qq = divmod(idx, 2)
                            blk = rnk + G * (2 * p_ + qq)
                            src = ag_out[r][p_][idx]
                            eng.dma_start(
                                hTv[:, :, blk * 128:(blk + 1) * 128],
                                src[:, 0:D].rearrange("p (c i) -> p c i", c=DC))
                        for idx in range(2 * G):
                            rnk, qq = divmod(idx, 2)
                            blk = rnk + G * (2 * p_ + qq)
                            src = ag_out[r][p_][idx]
                            eng.dma_start(hhi[:, blk * D:(blk + 1) * D],
                                          src[:, D:D + D // 2].bitcast(BF16))
                            eng.dma_start(hlo[:, blk * D:(blk + 1) * D],
                                          src[:, D + D // 2:PW].bitcast(BF16))

                    for p_ in range(NPH - 1):
                        _backs(p_, nc.sync)
                    _backs(NPH - 1, nc.gpsimd)

    nc.compile()
    return nc


# ------------------------------------------------------------------
# Host side
# ------------------------------------------------------------------

def _sigmoid(v):
    return 1.0 / (1.0 + math.exp(-float(v)))


def prep_inputs(cfg: Cfg, x, gain, bias, log_mix, log_momentum, log_scale):
    """Build the per-core input maps (numpy)."""
    P, D, T, DC, S, G, R = cfg.P, cfg.D, cfg.T, cfg.DC, cfg.S, cfg.G, cfg.R
    x = np.asarray(x, np.float32)
    gain = np.asarray(gain, np.float32)
    bias = np.asarray(bias, np.float32)
    mix = np.array([_sigmoid(v) for v in np.asarray(log_mix, np.float32)], np.float64)
    mom = _sigmoid(log_momentum)
    s = math.log1p(math.exp(float(log_scale))) + 0.01

    gm = np.ascontiguousarray(
        np.broadcast_to((gain * mix[:, None].astype(np.float32)).astype(np.float32)[:, None, :], (R, P, D)))
    bb = np.ascontiguousarray(np.broadcast_to(bias[:, None, :], (R, P, D)))
    scl = np.zeros((P, 4), np.float32)
    scl[:, 0] = mom
    scl[:, 1] = s * (1.0 - mom)
    scl[:, 2] = s * mom
    scl[:, 3] = 1.0 - mom
    idf = np.eye(128, dtype=np.float32)
    idb = np.eye(128, dtype=BF16_NP)

    in_maps = []
    for c in range(cfg.n_cores):
        b, cc = divmod(c, G)
        blocks = [cc + G * k for k in range(S)]
        h0 = x[b]  # [T, D]
        hT0 = np.ascontiguousarray(
            h0.T.reshape(DC, 128, T).transpose(1, 0, 2)).reshape(128, DC * T)
        hhi_f = h0.astype(BF16_NP)
        hlo_f = (h0 - hhi_f.astype(np.float32)).astype(BF16_NP)
        hhi0 = np.ascontiguousarray(
            hhi_f.reshape(cfg.NBLK, 128, D).transpose(1, 0, 2)).reshape(128, cfg.NBLK * D)
        hlo0 = np.ascontiguousarray(
            hlo_f.reshape(cfg.NBLK, 128, D).transpose(1, 0, 2)).reshape(128, cfg.NBLK * D)
        hb = h0.reshape(cfg.NBLK, 128, D)[blocks]  # [S,128,D]
        myh0 = np.ascontiguousarray(hb.transpose(1, 0, 2)).reshape(128, S * D)
        hTm0 = np.ascontiguousarray(
            hb.reshape(S, 128, DC, 128).transpose(3, 0, 2, 1)).reshape(128, S * D)
        msk = np.zeros((P, cfg.MTOT), BF16_NP)
        for k in range(S):
            W = cfg.g[k] * 128
            rowid = blocks[k] * 128 + np.arange(128)  # [128]
            j = np.arange(W)
            m = np.where(j[None, :] <= rowid[:, None], 0.0, NEG_MASK).astype(BF16_NP)
            msk[:, cfg.OFF[k]:cfg.OFF[k] + W] = m
        w2 = np.zeros((P, R * S), np.float32)
        for r in range(R):
            for k in range(S):
                cnt = np.minimum(blocks[k] * 128 + np.arange(128) + 1, K_SCHEDULE[r])
                w2[:, r * S + k] = ((1.0 - mix[r]) / (mix[r] * cnt)).astype(np.float32)
        in_maps.append({
            "i_hT": hT0, "i_hhi": hhi0, "i_hlo": hlo0, "i_myh": myh0,
            "i_hTm": hTm0, "i_msk": msk, "i_xs": (s * myh0.astype(np.float64)).astype(np.float32),
            "i_gm": gm, "i_bb": bb, "i_w2": w2, "i_sc": scl,
            "i_idf": idf, "i_idb": idb,
        })
    return in_maps


def assemble_output(cfg: Cfg, results, dtype=np.float32):
    """results: list (per core) of {'o_out': [S,128,D]} -> full [B,T,D]."""
    out = np.zeros((cfg.B, cfg.T, cfg.D), dtype)
    for c in range(cfg.n_cores):
        b, cc = divmod(c, cfg.G)
        o = results[c]["o_out"]
        for k in range(cfg.S):
            blk = cc + cfg.G * k
            out[b, blk * 128:(blk + 1) * 128] = o[k]
    return out


_PROGRAM_CACHE = {}


def _get_program(cfg: Cfg):
    key = (cfg.B, cfg.T, cfg.D, cfg.G, cfg.S)
    if key not in _PROGRAM_CACHE:
        _PROGRAM_CACHE[key] = build_program(cfg)
    return _PROGRAM_CACHE[key]


def run(cfg: Cfg, inputs: dict, trace: bool = False):
    nc = _get_program(cfg)
    in_maps = prep_inputs(cfg, **inputs)
    res = bass_utils.run_bass_kernel_spmd(
        nc, in_maps, list(range(cfg.n_cores)), trace=trace)
    out = assemble_output(cfg, res.results)
    return out, res


def kernel(**inputs) -> np.ndarray:
    cfg = Cfg()  # B=2, T=2048, D=1024, 8 cores
    out, _ = run(cfg, inputs)
    return out.astype(np.float32)



# revision 9
# speedup vs baseline: 1.2929x; 1.2929x over previous
"""Trainium2 Bass kernel for nn_DGN6 (gnn_message_passing).

Reference computation (per batch element, 3 rounds with K = 4, 8, 16):
    S = h @ h.T; causal top-K neighbors per row; msg = masked mean of
    neighbor rows; h = mom*h + (1-mom)*gelu((mix*h + (1-mix)*msg)*gain + bias)
Output: (h - x) * scale.

Distribution: data-parallel over B (2 batches), each batch's rows split
over 4 cores (8 cores total).  Core c handles batch c//4 and, within it,
4 row-blocks of 128 rows: blocks {cc + 4k, k=0..3} where cc = c%4 ("slot"
k holds block cc + 4k).  Every core runs an IDENTICAL instruction stream
(one SPMD program); per-core differences live entirely in input DATA
(causal masks, row data, per-row weights).

Key differences vs the first-pass kernel (1145us modeled):
  * Shared state h is kept in bf16 (hrows row-major + hT transposed), so
    score and aggregation matmuls run at 1 cycle/row on the PE (the fp32
    score matmuls were 4 cycles/row) and the inter-core payload per round
    is 2KB/row instead of 8KB/row.  Correctness gate is rel<2e-2;
    bf16 scores/agg measure ~3e-3.
  * AllGather payload carries only the row-major h' (bf16): receivers
    rebuild the transposed hT locally with PE transposes (cheap at bf16)
    right before each slot's scores, instead of shipping both layouts.
    Collective cost model is 15us + bytes/40GBps, so the 4x payload cut
    takes each of the 4 AllGathers from 225us to 67us.
  * gain == 1 / bias == 0 (the harness fill) is detected on the host and
    the affine multiply/add is skipped on device (a general path with the
    multiplies is compiled when the inputs are non-trivial).
  * Causal masks are loaded once (persistent in SBUF), not per round.

Per round, per slot:
  hT-build (rounds>0): PE-transpose this slot-group's 4 row-blocks from
          hrows into hT (psum->sbuf copies split over DVE/ACT).
  scores  S_blk = hTm[slot].T @ hT  (bf16 matmuls, fp32 PSUM accumulated
          over 8 d-chunks), masked-copied to SBUF with an additive causal
          mask (0 / -3e38).
  top-k   nc.vector.max (top-8) [+ match_replace + max again for K=16]
          -> threshold th = K-th largest; th' = max(th, -1e29).
  mask    M01 = (S_masked >= th') as bf16 0/1 (Pool engine) ->
          PE-transposed per 128-chunk to M01^T.
  agg     msg_raw = M01 @ hrows  (bf16, fp32 PSUM accumulate over the
          causal j-chunks).
  update  u = (msg_raw*w2 + h);  [general path: u = u*(gain*mix)+bias]
          h' = mom*h + (1-mom)*gelu(u)   (exact erf gelu on ACT engine)
          where w2 = (1-mix)/(mix*cnt) per row.
  layout  h' is cast to bf16 (ACT) and PE-transposed into hTm; the bf16
          rows are staged to DRAM for the phase AllGather.
Between rounds, h' rows are exchanged with two pipelined AllGathers per
round (groups of 4): phase 0 (slots 0-1) fires mid-round and overlaps
slots 2-3; phase 1 fires at the round boundary and overlaps the next
round's slots 0-1, which only read phase-0 blocks.  Back-DMAs write
hrows (2 strided DMAs per phase); hT is rebuilt lazily per slot.
Round 3 folds the momentum update and the final (h-x)*scale directly
into the output rows.

All scalar parameters (sigmoid/softplus of the inputs) are applied on the
host into small input tensors, so the device program depends only on
shapes (and the gain/bias triviality flag).
"""

import math
import numpy as np

import concourse.bacc as bacc
import concourse.bass as bass
import concourse.mybir as mybir
import concourse.tile as tile
from concourse import bass_utils
from concourse.alu_op_type import AluOpType

F32 = mybir.dt.float32
F32R = mybir.dt.float32r
BF16 = mybir.dt.bfloat16
AF = mybir.ActivationFunctionType
BF16_NP = mybir.dt.np(BF16)

NEG_MASK = -3.0e38  # additive causal mask value (bf16-representable)
NEG_CLAMP = -1.0e29  # threshold clamp: above mask, below any real score

K_SCHEDULE = (4, 8, 16)


class Cfg:
    def __init__(self, B=2, T=2048, D=1024, G=4, S=4, bf16=False, affine=False):
        self.B, self.T, self.D, self.G, self.S = B, T, D, G, S
        self.bf16 = bf16
        self.affine = affine  # general gain/bias path
        self.P = 128
        self.DC = D // 128          # d-chunks
        self.NBLK = G * S           # row blocks per batch
        assert self.NBLK * 128 == T
        self.n_cores = B * G
        self.R = len(K_SCHEDULE)
        # slot k covers j-chunks [0, g(k)); block of core cc in slot k is cc + G*k
        self.g = [G * (k + 1) for k in range(S)]
        self.OFF = [128 * sum(self.g[:k]) for k in range(S)]  # mask free-dim offsets
        self.MTOT = 128 * sum(self.g)
        self.groups = [list(range(b * G, (b + 1) * G)) for b in range(B)]


def build_program(cfg: Cfg):
    """Build the single SPMD Bass/Tile program (identical on all cores)."""
    nc = bacc.Bacc(
        "TRN2", target_bir_lowering=False, debug=False,
        num_devices=cfg.n_cores,
    )
    P, D, T, DC, S, G, R = cfg.P, cfg.D, cfg.T, cfg.DC, cfg.S, cfg.G, cfg.R
    # CD: dtype of the shared h state and all matmul operands.  Plain fp32
    # (4 PE cycles/row) is required for exactness: f32r (1 cycle/row) rounds
    # operands to ~10-bit mantissa on real HW, which flips top-k selections
    # vs the fp32 reference (measured 3.6e-2 rel, over the 2e-2 gate); bf16
    # is worse still (6e-2).  The score/agg matmuls are hidden under the
    # AllGather chain, so the 4x PE cost is mostly free.
    CD = BF16 if cfg.bf16 else F32
    TD = BF16 if cfg.bf16 else F32

    def mmcast(ap):
        return ap

    # ---- I/O ----
    i_hT = nc.dram_tensor("i_hT", [P, DC * T], CD, kind="ExternalInput")
    i_hr = nc.dram_tensor("i_hr", [P, cfg.NBLK * D], CD, kind="ExternalInput")
    i_myh = nc.dram_tensor("i_myh", [P, S * D], F32, kind="ExternalInput")
    i_hTm = nc.dram_tensor("i_hTm", [P, S * D], CD, kind="ExternalInput")
    i_msk = nc.dram_tensor("i_msk", [P, cfg.MTOT], BF16, kind="ExternalInput")
    i_xs = nc.dram_tensor("i_xs", [P, S * D], F32, kind="ExternalInput")
    i_w2 = nc.dram_tensor("i_w2", [P, R * S], F32, kind="ExternalInput")
    # per-partition scalar params: col 0 = mom, 1 = s*(1-mom), 2 = s*mom,
    # 3 = 1-mom, 4+r = mix_r (gelu input scale on the trivial-affine path)
    i_sc = nc.dram_tensor("i_sc", [P, 8], F32, kind="ExternalInput")
    i_idc = nc.dram_tensor("i_idc", [P, 128], CD, kind="ExternalInput")
    i_idf = nc.dram_tensor("i_idf", [P, 128], F32, kind="ExternalInput")
    i_idb = nc.dram_tensor("i_idb", [P, 128], BF16, kind="ExternalInput")
    if cfg.affine:
        i_gm = nc.dram_tensor("i_gm", [R, P, D], F32, kind="ExternalInput")
        i_bb = nc.dram_tensor("i_bb", [R, P, D], F32, kind="ExternalInput")
    o_out = nc.dram_tensor("o_out", [S, P, D], F32, kind="ExternalOutput")

    NH = D // 512  # 512-wide halves of D
    with tile.TileContext(nc) as tc:
        with (
            tc.tile_pool(name="const", bufs=1) as const,
            tc.tile_pool(name="work", bufs=2) as work,
            tc.tile_pool(name="psum", bufs=2, space="PSUM") as psum,
            tc.tile_pool(name="dram", bufs=1, space="DRAM") as dram,
        ):
            # ---- persistent state ----
            hT = const.tile([P, DC * T], CD, name="hT")
            hrows = const.tile([P, cfg.NBLK * D], CD, name="hrows")
            myh = const.tile([P, S * D], F32, name="myh")
            hTm = const.tile([P, S * D], CD, name="hTm")
            w2t = const.tile([P, R * S], F32, name="w2t")
            sct = const.tile([P, 8], F32, name="sct")
            idc = const.tile([P, 128], CD, name="idc")
            idf = const.tile([P, 128], F32, name="idf")
            idb = const.tile([P, 128], BF16, name="idb")

            hTv = hT.rearrange("p (c j) -> p c j", c=DC)
            iTv = i_hT[:].rearrange("p (c j) -> p c j", c=DC)

            # initial loads in strict first-use order: slot-0's scores need
            # hTm[0], the hT j-window [0,512), the identity, its mask and its
            # agg rows; later slots' state rides the Pool queue in need-order.
            j1_0 = cfg.g[0]
            nc.sync.dma_start(hTm[:, 0:D], i_hTm[:, 0:D])
            nc.sync.dma_start(hTv[:, :, 0:j1_0 * 128], iTv[:, :, 0:j1_0 * 128])
            nc.sync.dma_start(idc[:], i_idc[:])
            nc.sync.dma_start(idb[:], i_idb[:])
            nc.sync.dma_start(idf[:], i_idf[:])
            nc.sync.dma_start(hrows[:, 0:j1_0 * D], i_hr[:, 0:j1_0 * D])
            nc.sync.dma_start(sct[:], i_sc[:])
            nc.sync.dma_start(w2t[:], i_w2[:])
            nc.sync.dma_start(hTm[:, D:S * D], i_hTm[:, D:S * D])
            nc.sync.dma_start(myh[:], i_myh[:])
            for k in range(1, S):  # bulk, in the order later slots need it
                j0, j1 = cfg.g[k - 1], cfg.g[k]
                nc.gpsimd.dma_start(hTv[:, :, j0 * 128:j1 * 128],
                                    iTv[:, :, j0 * 128:j1 * 128])
                nc.gpsimd.dma_start(hrows[:, j0 * D:j1 * D], i_hr[:, j0 * D:j1 * D])

            ap_mom = sct[:, 0:1]
            ap_s1m = sct[:, 1:2]
            ap_sm = sct[:, 2:3]
            ap_1m = sct[:, 3:4]

            # per-round, per-phase AllGather buffers (DRAM), rows-only payload.
            NPH = S // 2  # phases per round
            ag_in = [[dram.tile([2, P, D], CD, name=f"ag_in{r}_{p}", tag=f"agi{r}_{p}")
                      for p in range(NPH)] for r in range(R - 1)]
            ag_out = [[dram.tile([2 * G, P, D], CD, name=f"ag_out{r}_{p}", tag=f"ago{r}_{p}")
                       for p in range(NPH)] for r in range(R - 1)]

            # alternate psum->sbuf copy engines to balance DVE/ACT load
            _alt = [0]

            def copy_out(dst, src):
                _alt[0] ^= 1
                if _alt[0]:
                    nc.vector.tensor_copy(dst, src)
                else:
                    nc.scalar.activation(dst, src, AF.Copy)

            for r in range(R):
                K = K_SCHEDULE[r]
                if cfg.affine:
                    gmt = work.tile([P, D], F32, tag="gmt", bufs=1, name=f"gmt{r}")
                    nc.scalar.dma_start(gmt[:], i_gm[r])
                    bbt = work.tile([P, D], F32, tag="bbt", bufs=1, name=f"bbt{r}")
                    nc.scalar.dma_start(bbt[:], i_bb[r])

                # propagation of slot k-1 is deferred until slot k's scores are
                # queued (the h'-transposes wait on the DVE/ACT elementwise
                # chain and would otherwise stall the next slot's matmuls on
                # the strict PE FIFO).
                pending_prop = None

                for k in range(S):
                    g = cfg.g[k]
                    W = g * 128
                    msk = work.tile([P, W], BF16, tag="msk", bufs=1, name=f"msk_{r}_{k}")
                    nc.scalar.dma_start(msk[:], i_msk[:, cfg.OFF[k]:cfg.OFF[k] + W])
                    # ---- rebuild hT for this slot-group's blocks (r>0) ----
                    if r > 0:
                        for blk in range(G * k, G * (k + 1)):
                            for half in range(2):
                                ptr = psum.tile([P, 512], CD, tag="pt", bufs=2,
                                                name=f"ptr_{r}_{blk}_{half}")
                                for i4 in range(4):
                                    dc = half * 4 + i4
                                    nc.tensor.transpose(
                                        ptr[:, i4 * 128:(i4 + 1) * 128],
                                        hrows[:, blk * D + dc * 128: blk * D + (dc + 1) * 128],
                                        idc[:])
                                dst = hTv[:, half * 4:half * 4 + 4,
                                          blk * 128:(blk + 1) * 128]
                                src = ptr[:].rearrange("p (c j) -> p c j", c=4)
                                copy_out(dst, src)
                    # ---- scores: S_blk = hTm[slot k].T @ hT[:, :W] ----
                    sc = work.tile([P, W], F32, tag="sc", bufs=2, name=f"sc_{r}_{k}")
                    for w0 in range(0, W, 512):
                        ps = psum.tile([P, 512], F32, tag="ps_sc", bufs=2,
                                       name=f"ps_{r}_{k}_{w0}")
                        for dc in range(DC):
                            nc.tensor.matmul(
                                ps[:],
                                mmcast(hTm[:, (k * DC + dc) * 128:(k * DC + dc + 1) * 128]),
                                mmcast(hT[:, dc * T + w0: dc * T + w0 + 512]),
                                start=(dc == 0), stop=(dc == DC - 1),
                            )
                        # masked copy PSUM -> SBUF: sc = S + mask  (mask 0 / -3e38)
                        nc.vector.scalar_tensor_tensor(
                            sc[:, w0:w0 + 512], ps[:], 1.0,
                            msk[:, w0:w0 + 512],
                            AluOpType.mult, AluOpType.add,
                        )
                    if pending_prop is not None:
                        pending_prop()
                        pending_prop = None
                    # ---- top-K threshold ----
                    mx = work.tile([P, 8], F32, tag="mx", name=f"mx_{r}_{k}")
                    nc.vector.max(out=mx[:], in_=sc[:])
                    if K <= 8:
                        th_src = mx[:, K - 1:K]
                    else:
                        scr = work.tile([P, W], F32, tag="sc", bufs=2, name=f"scr_{r}_{k}")
                        nc.vector.match_replace(out=scr[:], in_to_replace=mx[:],
                                                in_values=sc[:], imm_value=NEG_MASK)
                        mx2 = work.tile([P, 8], F32, tag="mx2", name=f"mx2_{r}_{k}")
                        nc.vector.max(out=mx2[:], in_=scr[:])
                        th_src = mx2[:, K - 9:K - 8]
                    th = work.tile([P, 1], F32, tag="th", name=f"th_{r}_{k}")
                    nc.vector.tensor_scalar_max(th[:], th_src, NEG_CLAMP)
                    # ---- 0/1 neighbor mask (Pool engine; CD dtype) ----
                    m01 = work.tile([P, W], BF16, tag="m01", bufs=1, name=f"m01_{r}_{k}")
                    nc.gpsimd.tensor_scalar(m01[:], sc[:], th[:], None, AluOpType.is_ge)
                    # ---- transpose mask chunks ----
                    mts = work.tile([P, W], CD, tag="mt", bufs=1, name=f"mt_{r}_{k}")
                    for jq in range(0, g, 4):
                        ptm = psum.tile([P, 512], BF16, tag="ptb", bufs=2,
                                        name=f"ptm_{r}_{k}_{jq}")
                        for i4 in range(4):
                            nc.tensor.transpose(
                                ptm[:, i4 * 128:(i4 + 1) * 128],
                                m01[:, (jq + i4) * 128:(jq + i4 + 1) * 128], idb[:])
                        copy_out(mts[:, jq * 128:(jq + 4) * 128], ptm[:])
                    # ---- aggregate: msg_raw = M01 @ hrows ----
                    pss = []
                    for h_ in range(NH):
                        pa = psum.tile([P, 512], F32, tag="ps_ag", bufs=2,
                                       name=f"pa_{r}_{k}_{h_}")
                        for jc in range(g):
                            nc.tensor.matmul(
                                pa[:], mmcast(mts[:, jc * 128:(jc + 1) * 128]),
                                mmcast(hrows[:, jc * D + h_ * 512: jc * D + h_ * 512 + 512]),
                                start=(jc == 0), stop=(jc == g - 1))
                        pss.append(pa)
                    # ---- elementwise update ----
                    w2ap = w2t[:, r * S + k: r * S + k + 1]
                    for h_ in range(NH):
                        sl = slice(k * D + h_ * 512, k * D + (h_ + 1) * 512)
                        hsl = slice(h_ * 512, (h_ + 1) * 512)
                        t1 = work.tile([P, 512], F32, tag="t1", name=f"t1_{r}_{k}_{h_}")
                        nc.vector.scalar_tensor_tensor(
                            t1[:], pss[h_][:], w2ap, myh[:, sl],
                            AluOpType.mult, AluOpType.add)
                        if cfg.affine:
                            nc.vector.tensor_mul(t1[:], t1[:], gmt[:, hsl])
                            nc.vector.tensor_add(t1[:], t1[:], bbt[:, hsl])
                        gl = work.tile([P, 512], F32, tag="gl", name=f"gl_{r}_{k}_{h_}")
                        if cfg.affine:
                            nc.scalar.activation(gl[:], t1[:], AF.Gelu)
                        else:
                            # u = (psum*w2 + h) * mix  (w2 carries the /mix)
                            nc.scalar.activation(gl[:], t1[:], AF.Gelu,
                                                 scale=sct[:, 4 + r:5 + r])
                        if r < R - 1:
                            nc.vector.tensor_scalar_mul(gl[:], gl[:], ap_1m)
                            # in-place: myh = mom*myh + (1-mom)*gelu
                            nc.vector.scalar_tensor_tensor(
                                myh[:, sl], myh[:, sl], ap_mom, gl[:],
                                AluOpType.mult, AluOpType.add)
                        else:
                            xst = work.tile([P, 512], F32, tag="xst", bufs=1,
                                            name=f"xst_{r}_{k}_{h_}")
                            nc.sync.dma_start(xst[:], i_xs[:, sl])
                            # gl <- s*(1-mom)*gelu - s*x
                            nc.vector.scalar_tensor_tensor(
                                gl[:], gl[:], ap_s1m, xst[:],
                                AluOpType.mult, AluOpType.subtract)
                            # t1 <- s*mom*h + gl
                            nc.vector.scalar_tensor_tensor(
                                t1[:], myh[:, sl], ap_sm, gl[:],
                                AluOpType.mult, AluOpType.add)
                            nc.sync.dma_start(o_out[k, :, hsl], t1[:])
                    # ---- propagate h' (transposed into hTm + staged rows) ----
                    if r < R - 1:
                        def _prop(kk=k, rr=r):
                            if cfg.bf16:
                                hi_t = work.tile([P, D], CD, tag="hi_t", bufs=1,
                                                 name=f"hi_{rr}_{kk}")
                                nc.scalar.activation(
                                    hi_t[:], myh[:, kk * D:(kk + 1) * D], AF.Copy)
                                src = hi_t[:]
                            else:
                                src = myh[:, kk * D:(kk + 1) * D]
                            for half in range(2):
                                pto = psum.tile([P, 512], TD, tag="pt", bufs=2,
                                                name=f"pto_{rr}_{kk}_{half}")
                                for i4 in range(4):
                                    dc = half * 4 + i4
                                    nc.tensor.transpose(
                                        pto[:, i4 * 128:(i4 + 1) * 128],
                                        src[:, dc * 128:(dc + 1) * 128],
                                        idc[:] if cfg.bf16 else idf[:])
                                copy_out(hTm[:, (kk * DC + half * 4) * 128:
                                              (kk * DC + half * 4 + 4) * 128], pto[:])
                            p_, q_ = divmod(kk, 2)
                            nc.sync.dma_start(ag_in[rr][p_][q_], src)
                            # fire the phase collective as soon as its 2 slots
                            # are staged; it only touches DRAM, so it overlaps
                            # the remaining slots' compute.
                            if q_ == 1 and p_ < NPH - 1:
                                nc.gpsimd.collective_compute(
                                    "AllGather", AluOpType.bypass,
                                    replica_groups=cfg.groups,
                                    ins=[ag_in[rr][p_].opt()],
                                    outs=[ag_out[rr][p_].opt()])
                        pending_prop = _prop

                if pending_prop is not None:
                    pending_prop()
                    pending_prop = None

                # ---- round boundary: last phase collective + back-DMAs ----
                # Emitted after the slot loop so Tile orders them after this
                # round's readers of hrows (they carry next round's state).
                if r < R - 1:
                    nc.gpsimd.collective_compute(
                        "AllGather", AluOpType.bypass, replica_groups=cfg.groups,
                        ins=[ag_in[r][NPH - 1].opt()], outs=[ag_out[r][NPH - 1].opt()])

                    def _backs(p_, eng):
                        # one strided DMA per (phase, q): ranks land contiguous.
                        # SBUF dst keeps the partition dim first; the DRAM src
                        # is permuted to match the dst iteration order.
                        srcv = ag_out[r][p_][:].rearrange(
                            "(rnk q) p d -> q p rnk d", q=2)
                        for q in range(2):
                            base = (2 * p_ + q) * G
                            dstv = hrows[:, base * D:(base + G) * D].rearrange(
                                "p (rnk d) -> p rnk d", rnk=G)
                            eng.dma_start(dstv, srcv[q])

                    _backs(0, nc.scalar if not cfg.affine else nc.sync)
                    _backs(1, nc.gpsimd)

    nc.compile()
    return nc


# ------------------------------------------------------------------
# Host side
# ------------------------------------------------------------------

def _sigmoid(v):
    return 1.0 / (1.0 + math.exp(-float(v)))


def prep_inputs(cfg: Cfg, x, gain, bias, log_mix, log_momentum, log_scale):
    """Build the per-core input maps (numpy)."""
    P, D, T, DC, S, G, R = cfg.P, cfg.D, cfg.T, cfg.DC, cfg.S, cfg.G, cfg.R
    cd_np = BF16_NP if cfg.bf16 else np.float32
    x = np.asarray(x, np.float32)
    gain = np.asarray(gain, np.float32)
    bias = np.asarray(bias, np.float32)
    mix = np.array([_sigmoid(v) for v in np.asarray(log_mix, np.float32)], np.float64)
    mom = _sigmoid(log_momentum)
    s = math.log1p(math.exp(float(log_scale))) + 0.01

    scl = np.zeros((P, 8), np.float32)
    scl[:, 0] = mom
    scl[:, 1] = s * (1.0 - mom)
    scl[:, 2] = s * mom
    scl[:, 3] = 1.0 - mom
    scl[:, 4:4 + R] = mix.astype(np.float32)[None, :]
    idc = np.eye(128, dtype=cd_np)

    common = {"i_sc": scl, "i_idc": idc, "i_idb": np.eye(128, dtype=BF16_NP),
              "i_idf": np.eye(128, dtype=np.float32)}
    if cfg.affine:
        common["i_gm"] = np.ascontiguousarray(np.broadcast_to(
            (gain * mix[:, None].astype(np.float32)).astype(np.float32)[:, None, :],
            (R, P, D)))
        common["i_bb"] = np.ascontiguousarray(np.broadcast_to(bias[:, None, :], (R, P, D)))

    in_maps = []
    for c in range(cfg.n_cores):
        b, cc = divmod(c, G)
        blocks = [cc + G * k for k in range(S)]
        h0 = x[b]  # [T, D]
        hT0 = np.ascontiguousarray(
            h0.T.reshape(DC, 128, T).transpose(1, 0, 2)).reshape(128, DC * T).astype(cd_np)
        hr0 = np.ascontiguousarray(
            h0.astype(cd_np).reshape(cfg.NBLK, 128, D).transpose(1, 0, 2)).reshape(128, cfg.NBLK * D)
        hb = h0.reshape(cfg.NBLK, 128, D)[blocks]  # [S,128,D]
        myh0 = np.ascontiguousarray(hb.transpose(1, 0, 2)).reshape(128, S * D)
        hTm0 = np.ascontiguousarray(
            hb.reshape(S, 128, DC, 128).transpose(3, 0, 2, 1)).reshape(128, S * D).astype(cd_np)
        msk = np.zeros((P, cfg.MTOT), BF16_NP)
        for k in range(S):
            W = cfg.g[k] * 128
            rowid = blocks[k] * 128 + np.arange(128)  # [128]
            j = np.arange(W)
            m = np.where(j[None, :] <= rowid[:, None], 0.0, NEG_MASK).astype(BF16_NP)
            msk[:, cfg.OFF[k]:cfg.OFF[k] + W] = m
        w2 = np.zeros((P, R * S), np.float32)
        for r in range(R):
            for k in range(S):
                cnt = np.minimum(blocks[k] * 128 + np.arange(128) + 1, K_SCHEDULE[r])
                w2[:, r * S + k] = ((1.0 - mix[r]) / (mix[r] * cnt)).astype(np.float32)
        in_maps.append(dict(common,
                            i_hT=hT0, i_hr=hr0, i_myh=myh0, i_hTm=hTm0,
                            i_msk=msk, i_xs=(s * myh0.astype(np.float64)).astype(np.float32),
                            i_w2=w2))
    return in_maps


def assemble_output(cfg: Cfg, results, dtype=np.float32):
    """results: list (per core) of {'o_out': [S,128,D]} -> full [B,T,D]."""
    out = np.zeros((cfg.B, cfg.T, cfg.D), dtype)
    for c in range(cfg.n_cores):
        b, cc = divmod(c, cfg.G)
        o = results[c]["o_out"]
        for k in range(cfg.S):
            blk = cc + cfg.G * k
            out[b, blk * 128:(blk + 1) * 128] = o[k]
    return out


_PROGRAM_CACHE = {}


def _get_program(cfg: Cfg):
    key = (cfg.B, cfg.T, cfg.D, cfg.G, cfg.S, cfg.bf16, cfg.affine)
    if key not in _PROGRAM_CACHE:
        _PROGRAM_CACHE[key] = build_program(cfg)
    return _PROGRAM_CACHE[key]


def run(cfg: Cfg, inputs: dict, trace: bool = False):
    nc = _get_program(cfg)
    in_maps = prep_inputs(cfg, **inputs)
    res = bass_utils.run_bass_kernel_spmd(
        nc, in_maps, list(range(cfg.n_cores)), trace=trace)
    out = assemble_output(cfg, res.results)
    return out, res


def kernel(**inputs) -> np.ndarray:
    trivial = (np.all(np.asarray(inputs["gain"]) == 1.0)
               and np.all(np.asarray(inputs["bias"]) == 0.0))
    cfg = Cfg(affine=not trivial)  # B=2, T=2048, D=1024, 8 cores
    out, _ = run(cfg, inputs)
    return out.astype(np.float32)


# revision 24
# speedup vs baseline: 1.8634x; 1.4412x over previous
"""Trainium2 Bass kernel for nn_DGN6 (gnn_message_passing).

Reference computation (per batch element, 3 rounds with K = 4, 8, 16):
    S = h @ h.T; causal top-K neighbors per row; msg = masked mean of
    neighbor rows; h = mom*h + (1-mom)*gelu((mix*h + (1-mix)*msg)*gain + bias)
Output: (h - x) * scale.

Distribution: data-parallel over B (2 batches), each batch's rows split
over 4 cores (8 cores total).  Core c handles batch c//4 and, within it,
4 row-blocks of 128 rows: blocks {cc + 4k, k=0..3} where cc = c%4 ("slot"
k holds block cc + 4k).  Every core runs an IDENTICAL instruction stream
(one SPMD program); per-core differences live entirely in input DATA
(causal masks, row data, per-row weights).

Numerics (all learned the hard way, measured on HW):
  * Everything stays fp32.  The reference's top-K selection sits on a
    cliff: quantizing h to bf16 (6e-2), f32r/tf32 (3.6e-2), or even
    bf16+fp8-residual (5e-2) flips near-tied selections and single flips
    cost ~0.05 absolute error -- over the 2e-2 gate.  fp32 measures 3e-7.
  * fp32 matmuls cost 4 PE cycles/row, but the timeline is bound by the
    AllGather pipeline (cost model: 15us + out_bytes/40GBps, serialized
    per core), so most of the PE cost hides under it.

Layout/dataflow per round, per slot:
  hT-build (rounds>0) PE-transposes hrows blocks into hT right before
  the score strips that need them; scores accumulate 512-wide strips
  (8 d-chunk matmuls each) into PSUM, masked-copied to SBUF with an
  additive causal mask (0/-3e38, per-strip streamed tiles); top-K via
  vector.max (+match_replace+max for K=16) -> threshold; M01 = (S>=th)
  bf16; M01 chunks PE-transposed 4-at-a-time through a rotating tile and
  immediately consumed by the aggregation matmuls (fp32, PSUM-accumulated
  over causal j-chunks); update u = (msg*w2 + h), h' = mom*h +
  (1-mom)*gelu(u*mix) with mix applied via the ACT engine's input scale
  (w2 = (1-mix)/(mix*cnt) per row); gain==1/bias==0 is detected on the
  host (the general affine path is compiled otherwise).  Round 3 folds
  momentum and (h-x)*scale into the output.

Inter-core exchange (the critical path): rows-only fp32 payload, per
boundary phases {0},{1},{2,3} then {0,1},{2},{3}, each AllGather fired
the moment its slots' updates are staged.  The phase schedule keeps the
collective pipeline 100%% busy from ~38us to ~547us (zero gaps), with
each round's compute overlapped under it:
  * score strips over already-arrived blocks are emitted before rebuilds
    of still-in-flight blocks (slot 3's early strips run during the
    phase waits);
  * back-DMAs (collective -> hrows) are deferred into the NEXT round's
    schedule at exactly the hT-build that consumes them: emitted any
    earlier they wait on their collective while blocking later traffic
    (mask loads, staging) on the same DMA queue;
  * scheduler fences (tc.no_sync_barrier) at round boundaries and before
    phase-gated rebuilds stop the static scheduler from hoisting
    collective-gated work ahead of ready work in the per-engine queues
    (priority inversions measured 30-90us each);
  * NOTHING computes on the Pool queue: collectives live there, and any
    instruction queued behind one waits out its full modeled duration.

All scalar parameters (sigmoid/softplus of the inputs) are applied on
the host into small input tensors, so the device program depends only
on shapes (and the gain/bias triviality flag).
"""

import math
import numpy as np

import concourse.bacc as bacc
import concourse.bass as bass
import concourse.mybir as mybir
import concourse.tile as tile
from concourse import bass_utils
from concourse.alu_op_type import AluOpType

F32 = mybir.dt.float32
F32R = mybir.dt.float32r
BF16 = mybir.dt.bfloat16
AF = mybir.ActivationFunctionType
BF16_NP = mybir.dt.np(BF16)

NEG_MASK = -3.0e38  # additive causal mask value (bf16-representable)
NEG_CLAMP = -1.0e29  # threshold clamp: above mask, below any real score

K_SCHEDULE = (4, 8, 16)


class Cfg:
    def __init__(self, B=2, T=2048, D=1024, G=4, S=4, bf16=False, affine=False):
        self.B, self.T, self.D, self.G, self.S = B, T, D, G, S
        self.bf16 = bf16
        self.affine = affine  # general gain/bias path
        self.P = 128
        self.DC = D // 128          # d-chunks
        self.NBLK = G * S           # row blocks per batch
        assert self.NBLK * 128 == T
        self.n_cores = B * G
        self.R = len(K_SCHEDULE)
        # slot k covers j-chunks [0, g(k)); block of core cc in slot k is cc + G*k
        self.g = [G * (k + 1) for k in range(S)]
        self.OFF = [128 * sum(self.g[:k]) for k in range(S)]  # mask free-dim offsets
        self.MTOT = 128 * sum(self.g)
        self.groups = [list(range(b * G, (b + 1) * G)) for b in range(B)]


def build_program(cfg: Cfg):
    """Build the single SPMD Bass/Tile program (identical on all cores)."""
    nc = bacc.Bacc(
        "TRN2", target_bir_lowering=False, debug=False,
        num_devices=cfg.n_cores,
    )
    P, D, T, DC, S, G, R = cfg.P, cfg.D, cfg.T, cfg.DC, cfg.S, cfg.G, cfg.R
    # CD: dtype of the shared h state and all matmul operands.  Plain fp32
    # (4 PE cycles/row) is required for exactness: f32r (1 cycle/row) rounds
    # operands to ~10-bit mantissa on real HW, which flips top-k selections
    # vs the fp32 reference (measured 3.6e-2 rel, over the 2e-2 gate); bf16
    # is worse still (6e-2).  The score/agg matmuls are hidden under the
    # AllGather chain, so the 4x PE cost is mostly free.
    CD = BF16 if cfg.bf16 else F32
    TD = BF16 if cfg.bf16 else F32

    def mmcast(ap):
        return ap

    # ---- I/O ----
    i_hT = nc.dram_tensor("i_hT", [P, DC * T], CD, kind="ExternalInput")
    i_hr = nc.dram_tensor("i_hr", [P, cfg.NBLK * D], CD, kind="ExternalInput")
    i_myh = nc.dram_tensor("i_myh", [P, S * D], F32, kind="ExternalInput")
    i_hTm = nc.dram_tensor("i_hTm", [P, S * D], CD, kind="ExternalInput")
    i_msk = nc.dram_tensor("i_msk", [P, cfg.MTOT], BF16, kind="ExternalInput")
    i_xs = nc.dram_tensor("i_xs", [P, S * D], F32, kind="ExternalInput")
    i_w2 = nc.dram_tensor("i_w2", [P, R * S], F32, kind="ExternalInput")
    # per-partition scalar params: col 0 = mom, 1 = s*(1-mom), 2 = s*mom,
    # 3 = 1-mom, 4+r = mix_r (gelu input scale on the trivial-affine path)
    i_sc = nc.dram_tensor("i_sc", [P, 8], F32, kind="ExternalInput")
    i_idc = nc.dram_tensor("i_idc", [P, 128], CD, kind="ExternalInput")
    i_idf = nc.dram_tensor("i_idf", [P, 128], F32, kind="ExternalInput")
    i_idb = nc.dram_tensor("i_idb", [P, 128], BF16, kind="ExternalInput")
    if cfg.affine:
        i_gm = nc.dram_tensor("i_gm", [R, P, D], F32, kind="ExternalInput")
        i_bb = nc.dram_tensor("i_bb", [R, P, D], F32, kind="ExternalInput")
    o_out = nc.dram_tensor("o_out", [S, P, D], F32, kind="ExternalOutput")

    NH = D // 512  # 512-wide halves of D
    with tile.TileContext(nc) as tc:
        with (
            tc.tile_pool(name="const", bufs=1) as const,
            tc.tile_pool(name="work", bufs=2) as work,
            tc.tile_pool(name="psum", bufs=2, space="PSUM") as psum,
            tc.tile_pool(name="dram", bufs=1, space="DRAM") as dram,
        ):
            # ---- persistent state ----
            hT = const.tile([P, DC * T], CD, name="hT")
            hrows = const.tile([P, cfg.NBLK * D], CD, name="hrows")
            myh = const.tile([P, S * D], F32, name="myh")
            hTm = const.tile([P, S * D], CD, name="hTm")
            w2t = const.tile([P, R * S], F32, name="w2t")
            sct = const.tile([P, 8], F32, name="sct")
            idc = const.tile([P, 128], CD, name="idc")
            idf = const.tile([P, 128], F32, name="idf")
            idb = const.tile([P, 128], BF16, name="idb")

            hTv = hT.rearrange("p (c j) -> p c j", c=DC)
            iTv = i_hT[:].rearrange("p (c j) -> p c j", c=DC)

            # initial loads in strict first-use order: slot-0's scores need
            # hTm[0], the hT j-window [0,512), the identity, its mask and its
            # agg rows; later slots' state rides the Pool queue in need-order.
            j1_0 = cfg.g[0]
            nc.sync.dma_start(hTm[:, 0:D], i_hTm[:, 0:D])
            # slot-0's first score matmul needs only chunk dc=0 of the hT
            # window; per-dc loads let the PE start ~7us earlier.
            for dc in range(DC):
                nc.sync.dma_start(hTv[:, dc:dc + 1, 0:j1_0 * 128],
                                  iTv[:, dc:dc + 1, 0:j1_0 * 128])
            nc.sync.dma_start(idc[:], i_idc[:])
            nc.sync.dma_start(idb[:], i_idb[:])
            nc.sync.dma_start(idf[:], i_idf[:])
            nc.sync.dma_start(hrows[:, 0:j1_0 * D], i_hr[:, 0:j1_0 * D])
            nc.sync.dma_start(sct[:], i_sc[:])
            nc.sync.dma_start(w2t[:], i_w2[:])
            nc.sync.dma_start(hTm[:, D:S * D], i_hTm[:, D:S * D])
            nc.sync.dma_start(myh[:], i_myh[:])
            for k in range(1, S):  # bulk, in the order later slots need it
                j0, j1 = cfg.g[k - 1], cfg.g[k]
                nc.gpsimd.dma_start(hTv[:, :, j0 * 128:j1 * 128],
                                    iTv[:, :, j0 * 128:j1 * 128])
                nc.gpsimd.dma_start(hrows[:, j0 * D:j1 * D], i_hr[:, j0 * D:j1 * D])

            ap_mom = sct[:, 0:1]
            ap_s1m = sct[:, 1:2]
            ap_sm = sct[:, 2:3]
            ap_1m = sct[:, 3:4]

            # per-round, per-phase AllGather buffers (DRAM), rows-only payload.
            # Boundary 0 ships slots {0,1} then {2,3}; boundary 1 (feeding the
            # final round) splits the tail phases so the last round's slot-2/3
            # chains start as soon as their own blocks arrive.
            PHS = [[[0], [1], [2, 3]], [[0, 1], [2], [3]]][:R - 1]
            ag_in = [[dram.tile([len(ph), P, D], CD, name=f"ag_in{r}_{p}", tag=f"agi{r}_{p}")
                      for p, ph in enumerate(PHS[r])] for r in range(R - 1)]
            ag_out = [[dram.tile([len(ph) * G, P, D], CD, name=f"ag_out{r}_{p}", tag=f"ago{r}_{p}")
                       for p, ph in enumerate(PHS[r])] for r in range(R - 1)]
            # slot -> (phase, q) per boundary
            PQ = [{k: (p, q) for p, ph in enumerate(phs) for q, k in enumerate(ph)}
                  for phs in PHS]

            # alternate psum->sbuf copy engines to balance DVE/ACT load
            _alt = [0]
            pending_backs = [{}]  # slot -> deferred back-DMA emitter

            def copy_out(dst, src):
                _alt[0] ^= 1
                if _alt[0]:
                    nc.vector.tensor_copy(dst, src)
                else:
                    nc.scalar.activation(dst, src, AF.Copy)

            for r in range(R):
                K = K_SCHEDULE[r]
                if cfg.affine:
                    gmt = work.tile([P, D], F32, tag="gmt", bufs=1, name=f"gmt{r}")
                    nc.scalar.dma_start(gmt[:], i_gm[r])
                    bbt = work.tile([P, D], F32, tag="bbt", bufs=1, name=f"bbt{r}")
                    nc.scalar.dma_start(bbt[:], i_bb[r])

                # propagation of slot k-1 is deferred until slot k's scores are
                # queued (the h'-transposes wait on the DVE/ACT elementwise
                # chain and would otherwise stall the next slot's matmuls on
                # the strict PE FIFO).
                pending_prop = None

                def build_blocks(b0, b1, rr=r):
                    # PE-transpose hrows blocks [b0,b1) into hT columns
                    for blk in range(b0, b1):
                        for half in range(2):
                            ptr = psum.tile([P, 512], CD, tag="pt", bufs=2,
                                            name=f"ptr_{rr}_{blk}_{half}")
                            for i4 in range(4):
                                dc = half * 4 + i4
                                nc.tensor.transpose(
                                    ptr[:, i4 * 128:(i4 + 1) * 128],
                                    hrows[:, blk * D + dc * 128: blk * D + (dc + 1) * 128],
                                    idc[:])
                            dst = hTv[:, half * 4:half * 4 + 4,
                                      blk * 128:(blk + 1) * 128]
                            src = ptr[:].rearrange("p (c j) -> p c j", c=4)
                            copy_out(dst, src)

                scs = {}

                def ensure_sc(kk, rr=r):
                    if kk not in scs:
                        W = cfg.g[kk] * 128
                        scs[kk] = work.tile([P, W], F32, tag="sc", bufs=2,
                                            name=f"sc_{rr}_{kk}")

                def score_strips(kk, w0_list, rr=r):
                    ensure_sc(kk)
                    for w0 in w0_list:
                        # per-strip causal mask chunk (small rotating bufs)
                        mskt = work.tile([P, 512], BF16, tag="msk", bufs=3,
                                         name=f"msk_{rr}_{kk}_{w0}")
                        nc.scalar.dma_start(
                            mskt[:], i_msk[:, cfg.OFF[kk] + w0:cfg.OFF[kk] + w0 + 512])
                        ps = psum.tile([P, 512], F32, tag="ps_sc", bufs=2,
                                       name=f"ps_{rr}_{kk}_{w0}")
                        for dc in range(DC):
                            nc.tensor.matmul(
                                ps[:],
                                mmcast(hTm[:, (kk * DC + dc) * 128:(kk * DC + dc + 1) * 128]),
                                mmcast(hT[:, dc * T + w0: dc * T + w0 + 512]),
                                start=(dc == 0), stop=(dc == DC - 1),
                            )
                        # masked copy PSUM -> SBUF: sc = S + mask (0 / -3e38)
                        nc.vector.scalar_tensor_tensor(
                            scs[kk][:, w0:w0 + 512], ps[:], 1.0, mskt[:],
                            AluOpType.mult, AluOpType.add,
                        )

                def slot_rest(k, rr=r):
                    """topk -> mask -> aggregate -> update -> defer prop."""
                    nonlocal pending_prop
                    g = cfg.g[k]
                    W = g * 128
                    sc = scs[k]
                    # ---- top-K threshold ----
                    mx = work.tile([P, 8], F32, tag="mx", name=f"mx_{rr}_{k}")
                    nc.vector.max(out=mx[:], in_=sc[:])
                    if K <= 8:
                        th_src = mx[:, K - 1:K]
                    else:
                        scr = work.tile([P, W], F32, tag="scr", bufs=1,
                                        name=f"scr_{rr}_{k}")
                        nc.vector.match_replace(out=scr[:], in_to_replace=mx[:],
                                                in_values=sc[:], imm_value=NEG_MASK)
                        mx2 = work.tile([P, 8], F32, tag="mx2", name=f"mx2_{rr}_{k}")
                        nc.vector.max(out=mx2[:], in_=scr[:])
                        th_src = mx2[:, K - 9:K - 8]
                    th = work.tile([P, 1], F32, tag="th", name=f"th_{rr}_{k}")
                    nc.vector.tensor_scalar_max(th[:], th_src, NEG_CLAMP)
                    # ---- 0/1 neighbor mask.  MUST NOT ride the Pool queue:
                    # collectives live there, and any instruction queued
                    # behind one waits for its (120us) completion. ----
                    m01 = work.tile([P, W], BF16, tag="m01", bufs=1, name=f"m01_{rr}_{k}")
                    nc.vector.tensor_scalar(m01[:], sc[:], th[:], None, AluOpType.is_ge)
                    # ---- transpose mask chunks + aggregate (interleaved):
                    # msg_raw = M01 @ hrows, accumulated 4 j-chunks at a time
                    # through a small rotating transposed-mask tile ----
                    pss = [psum.tile([P, 512], F32, tag="ps_ag", bufs=2,
                                     name=f"pa_{rr}_{k}_{h_}") for h_ in range(NH)]
                    for jq in range(0, g, 4):
                        ptm = psum.tile([P, 512], BF16, tag="ptb", bufs=2,
                                        name=f"ptm_{rr}_{k}_{jq}")
                        for i4 in range(4):
                            nc.tensor.transpose(
                                ptm[:, i4 * 128:(i4 + 1) * 128],
                                m01[:, (jq + i4) * 128:(jq + i4 + 1) * 128], idb[:])
                        mts = work.tile([P, 512], CD, tag="mt", bufs=2,
                                        name=f"mt_{rr}_{k}_{jq}")
                        copy_out(mts[:], ptm[:])
                        for h_ in range(NH):
                            for i4 in range(4):
                                jc = jq + i4
                                nc.tensor.matmul(
                                    pss[h_][:], mmcast(mts[:, i4 * 128:(i4 + 1) * 128]),
                                    mmcast(hrows[:, jc * D + h_ * 512: jc * D + h_ * 512 + 512]),
                                    start=(jc == 0), stop=(jc == g - 1))
                    # ---- elementwise update ----
                    w2ap = w2t[:, rr * S + k: rr * S + k + 1]
                    for h_ in range(NH):
                        sl = slice(k * D + h_ * 512, k * D + (h_ + 1) * 512)
                        hsl = slice(h_ * 512, (h_ + 1) * 512)
                        t1 = work.tile([P, 512], F32, tag="t1", name=f"t1_{rr}_{k}_{h_}")
                        nc.vector.scalar_tensor_tensor(
                            t1[:], pss[h_][:], w2ap, myh[:, sl],
                            AluOpType.mult, AluOpType.add)
                        if cfg.affine:
                            nc.vector.tensor_mul(t1[:], t1[:], gmt[:, hsl])
                            nc.vector.tensor_add(t1[:], t1[:], bbt[:, hsl])
                        gl = work.tile([P, 512], F32, tag="gl", name=f"gl_{rr}_{k}_{h_}")
                        if cfg.affine:
                            nc.scalar.activation(gl[:], t1[:], AF.Gelu)
                        else:
                            # u = (psum*w2 + h) * mix  (w2 carries the /mix)
                            nc.scalar.activation(gl[:], t1[:], AF.Gelu,
                                                 scale=sct[:, 4 + rr:5 + rr])
                        if rr < R - 1:
                            nc.vector.tensor_scalar_mul(gl[:], gl[:], ap_1m)
                            # in-place: myh = mom*myh + (1-mom)*gelu
                            nc.vector.scalar_tensor_tensor(
                                myh[:, sl], myh[:, sl], ap_mom, gl[:],
                                AluOpType.mult, AluOpType.add)
                        else:
                            xst = work.tile([P, 512], F32, tag="xst", bufs=1,
                                            name=f"xst_{rr}_{k}_{h_}")
                            nc.sync.dma_start(xst[:], i_xs[:, sl])
                            # gl <- s*(1-mom)*gelu - s*x
                            nc.vector.scalar_tensor_tensor(
                                gl[:], gl[:], ap_s1m, xst[:],
                                AluOpType.mult, AluOpType.subtract)
                            # t1 <- s*mom*h + gl
                            nc.vector.scalar_tensor_tensor(
                                t1[:], myh[:, sl], ap_sm, gl[:],
                                AluOpType.mult, AluOpType.add)
                            nc.sync.dma_start(o_out[k, :, hsl], t1[:])
                    # ---- propagate h' (transposed into hTm + staged rows) ----
                    if rr < R - 1:
                        def _prop(kk=k):
                            if cfg.bf16:
                                hi_t = work.tile([P, D], CD, tag="hi_t", bufs=1,
                                                 name=f"hi_{rr}_{kk}")
                                nc.scalar.activation(
                                    hi_t[:], myh[:, kk * D:(kk + 1) * D], AF.Copy)
                                src = hi_t[:]
                            else:
                                src = myh[:, kk * D:(kk + 1) * D]
                            for half in range(2):
                                pto = psum.tile([P, 512], TD, tag="pt", bufs=2,
                                                name=f"pto_{rr}_{kk}_{half}")
                                for i4 in range(4):
                                    dc = half * 4 + i4
                                    nc.tensor.transpose(
                                        pto[:, i4 * 128:(i4 + 1) * 128],
                                        src[:, dc * 128:(dc + 1) * 128],
                                        idc[:] if cfg.bf16 else idf[:])
                                copy_out(hTm[:, (kk * DC + half * 4) * 128:
                                              (kk * DC + half * 4 + 4) * 128], pto[:])
                            p_, q_ = PQ[rr][kk]
                            nc.sync.dma_start(ag_in[rr][p_][q_], src)
                            # fire the phase collective as soon as all its
                            # slots are staged; it only touches DRAM, so it
                            # overlaps the remaining slots' compute.
                            if q_ == len(PHS[rr][p_]) - 1 and p_ < len(PHS[rr]) - 1:
                                nc.gpsimd.collective_compute(
                                    "AllGather", AluOpType.bypass,
                                    replica_groups=cfg.groups,
                                    ins=[ag_in[rr][p_].opt()],
                                    outs=[ag_out[rr][p_].opt()])
                        pending_prop = _prop

                def fire_prop():
                    nonlocal pending_prop
                    if pending_prop is not None:
                        with tc.high_priority():
                            pending_prop()
                        pending_prop = None

                # ---- the round's emission schedule.  The PE FIFO is strict,
                # so emission order = execution order: score strips over
                # already-built hT blocks are emitted before rebuilds whose
                # back-DMA may still be waiting on a phase collective, and
                # slot 3's early strips run during the phase-1 wait.
                if r == 0:
                    for k in range(S):
                        score_strips(k, range(0, cfg.g[k] * 128, 512))
                        fire_prop()
                        slot_rest(k)
                else:
                    # scheduler fence at the round boundary: without it the
                    # static scheduler interleaves this round's DVE/ACT ops
                    # before the previous round's tail updates in the per-
                    # engine queues, stalling the staging of the boundary
                    # collectives behind collective-gated work.
                    tc.no_sync_barrier()

                    def fire_backs(at_slot):
                        em = pending_backs[0].pop(at_slot, None)
                        if em is not None:
                            em()

                    fire_backs(0)
                    build_blocks(0, G)
                    score_strips(0, [0])
                    slot_rest(0)
                    score_strips(1, [0])
                    fire_backs(1)
                    build_blocks(G, 2 * G)
                    score_strips(1, [512])
                    fire_prop()  # prop(0): stages the next boundary's phase 0
                    slot_rest(1)
                    score_strips(2, [0, 512])
                    fire_prop()  # prop(1) EARLY: its staging must not queue
                    #              behind the phase-dependent strips below
                    score_strips(3, [0, 512])
                    tc.no_sync_barrier()  # keep the phase-gated rebuild below
                    #                       from being scheduled before the
                    #                       ready work above
                    fire_backs(2)
                    build_blocks(2 * G, 3 * G)
                    score_strips(2, [1024])
                    score_strips(3, [1024])
                    slot_rest(2)
                    fire_prop()  # prop(2) early, same reason
                    tc.no_sync_barrier()
                    fire_backs(3)
                    build_blocks(3 * G, 4 * G)
                    score_strips(3, [1536])
                    slot_rest(3)

                fire_prop()  # prop(3)

                # ---- round boundary: last phase collective + back-DMAs ----
                # Back-DMAs are emitted after the slot loop so Tile orders
                # them after this round's readers of hrows.  The fence keeps
                # the scheduler from hoisting them ahead of this round's tail
                # ops on the shared DMA queues (priority inversion: a back-DMA
                # WAR-waits on this round's aggregation, which itself needs a
                # psum copy queued behind that same back-DMA).
                if r < R - 1:
                    tc.no_sync_barrier()
                    nph = len(PHS[r])
                    nc.gpsimd.collective_compute(
                        "AllGather", AluOpType.bypass, replica_groups=cfg.groups,
                        ins=[ag_in[r][nph - 1].opt()], outs=[ag_out[r][nph - 1].opt()])

                    def _backs(p_, eng, rr=r):
                        # one strided DMA per (phase, q): ranks land contiguous.
                        # SBUF dst keeps the partition dim first; the DRAM src
                        # is permuted to match the dst iteration order.
                        nq = len(PHS[rr][p_])
                        srcv = ag_out[rr][p_][:].rearrange(
                            "(rnk q) p d -> q p rnk d", q=nq)
                        for q in range(nq):
                            base = PHS[rr][p_][q] * G
                            dstv = hrows[:, base * D:(base + G) * D].rearrange(
                                "p (rnk d) -> p rnk d", rnk=G)
                            eng.dma_start(dstv, srcv[q])

                    # ALL back-DMAs are deferred into the next round's
                    # schedule (sync queue), each emitted just before the
                    # hT rebuild that consumes it: emitted any earlier they
                    # sit on a DMA queue WAITING on their phase collective,
                    # blocking later traffic on that queue (mask loads,
                    # staging) that the next round needs much sooner.
                    pending_backs[0] = {
                        PHS[r][p_][0]: (lambda rr=r, pp=p_: _backs(pp, nc.sync, rr))
                        for p_ in range(nph)}

    nc.compile()
    return nc


# ------------------------------------------------------------------
# Host side
# ------------------------------------------------------------------

def _sigmoid(v):
    return 1.0 / (1.0 + math.exp(-float(v)))


def prep_inputs(cfg: Cfg, x, gain, bias, log_mix, log_momentum, log_scale):
    """Build the per-core input maps (numpy)."""
    P, D, T, DC, S, G, R = cfg.P, cfg.D, cfg.T, cfg.DC, cfg.S, cfg.G, cfg.R
    cd_np = BF16_NP if cfg.bf16 else np.float32
    x = np.asarray(x, np.float32)
    gain = np.asarray(gain, np.float32)
    bias = np.asarray(bias, np.float32)
    mix = np.array([_sigmoid(v) for v in np.asarray(log_mix, np.float32)], np.float64)
    mom = _sigmoid(log_momentum)
    s = math.log1p(math.exp(float(log_scale))) + 0.01

    scl = np.zeros((P, 8), np.float32)
    scl[:, 0] = mom
    scl[:, 1] = s * (1.0 - mom)
    scl[:, 2] = s * mom
    scl[:, 3] = 1.0 - mom
    scl[:, 4:4 + R] = mix.astype(np.float32)[None, :]
    idc = np.eye(128, dtype=cd_np)

    common = {"i_sc": scl, "i_idc": idc, "i_idb": np.eye(128, dtype=BF16_NP),
              "i_idf": np.eye(128, dtype=np.float32)}
    if cfg.affine:
        common["i_gm"] = np.ascontiguousarray(np.broadcast_to(
            (gain * mix[:, None].astype(np.float32)).astype(np.float32)[:, None, :],
            (R, P, D)))
        common["i_bb"] = np.ascontiguousarray(np.broadcast_to(bias[:, None, :], (R, P, D)))

    in_maps = []
    for c in range(cfg.n_cores):
        b, cc = divmod(c, G)
        blocks = [cc + G * k for k in range(S)]
        h0 = x[b]  # [T, D]
        hT0 = np.ascontiguousarray(
            h0.T.reshape(DC, 128, T).transpose(1, 0, 2)).reshape(128, DC * T).astype(cd_np)
        hr0 = np.ascontiguousarray(
            h0.astype(cd_np).reshape(cfg.NBLK, 128, D).transpose(1, 0, 2)).reshape(128, cfg.NBLK * D)
        hb = h0.reshape(cfg.NBLK, 128, D)[blocks]  # [S,128,D]
        myh0 = np.ascontiguousarray(hb.transpose(1, 0, 2)).reshape(128, S * D)
        hTm0 = np.ascontiguousarray(
            hb.reshape(S, 128, DC, 128).transpose(3, 0, 2, 1)).reshape(128, S * D).astype(cd_np)
        msk = np.zeros((P, cfg.MTOT), BF16_NP)
        for k in range(S):
            W = cfg.g[k] * 128
            rowid = blocks[k] * 128 + np.arange(128)  # [128]
            j = np.arange(W)
            m = np.where(j[None, :] <= rowid[:, None], 0.0, NEG_MASK).astype(BF16_NP)
            msk[:, cfg.OFF[k]:cfg.OFF[k] + W] = m
        w2 = np.zeros((P, R * S), np.float32)
        for r in range(R):
            for k in range(S):
                cnt = np.minimum(blocks[k] * 128 + np.arange(128) + 1, K_SCHEDULE[r])
                w2[:, r * S + k] = ((1.0 - mix[r]) / (mix[r] * cnt)).astype(np.float32)
        in_maps.append(dict(common,
                            i_hT=hT0, i_hr=hr0, i_myh=myh0, i_hTm=hTm0,
                            i_msk=msk, i_xs=(s * myh0.astype(np.float64)).astype(np.float32),
                            i_w2=w2))
    return in_maps


def assemble_output(cfg: Cfg, results, dtype=np.float32):
    """results: list (per core) of {'o_out': [S,128,D]} -> full [B,T,D]."""
    out = np.zeros((cfg.B, cfg.T, cfg.D), dtype)
    for c in range(cfg.n_cores):
        b, cc = divmod(c, cfg.G)
        o = results[c]["o_out"]
        for k in range(cfg.S):
            blk = cc + cfg.G * k
            out[b, blk * 128:(blk + 1) * 128] = o[k]
    return out


_PROGRAM_CACHE = {}


def _get_program(cfg: Cfg):
    key = (cfg.B, cfg.T, cfg.D, cfg.G, cfg.S, cfg.bf16, cfg.affine)
    if key not in _PROGRAM_CACHE:
        _PROGRAM_CACHE[key] = build_program(cfg)
    return _PROGRAM_CACHE[key]


def run(cfg: Cfg, inputs: dict, trace: bool = False):
    nc = _get_program(cfg)
    in_maps = prep_inputs(cfg, **inputs)
    res = bass_utils.run_bass_kernel_spmd(
        nc, in_maps, list(range(cfg.n_cores)), trace=trace)
    out = assemble_output(cfg, res.results)
    return out, res


def kernel(**inputs) -> np.ndarray:
    trivial = (np.all(np.asarray(inputs["gain"]) == 1.0)
               and np.all(np.asarray(inputs["bias"]) == 0.0))
    cfg = Cfg(affine=not trivial)  # B=2, T=2048, D=1024, 8 cores
    out, _ = run(cfg, inputs)
    return out.astype(np.float32)
